# revision 17
# baseline (speedup 1.0000x reference)
"""GQA prefill attention (B=2, S=2048, D=2048, H=32, KV=8, HD=64) on 8 trn2 cores.

Sharding: batch x kv-pair. Core c = (b = c//4, g = c%4) owns batch b,
q-heads [8g, 8g+8) and kv-heads {2g, 2g+1}; computes its partial of
out_b = attn_out @ wo_chunk; host sums the 4 partials per batch.

Per 512-seq stripe: projections (6 psum chunks: 4 q-head-pairs, K, V) with
RoPE via pair-swap permutation matmul; V transposed to [s, dh] on PE.
Attention qi-outer / pair-inner per 256-q block: both heads of a pair share
the kv head, so one K=64 score matmul with rhs=[Q_even|Q_odd] fills one psum
bank [128k, 256q_e | 256q_o]; exp on ACT; partially masked blocks multiplied
by exp(mask^T) tiles; one PV matmul lhsT=[V|1] rhs=P gives OT[dh|rowsum, 512]
accumulated over k-blocks. Normalization: OT+rowsum copied to SBUF (frees
the bank), approx-reciprocal on DVE, partition-broadcast on GpSimd, two
DVE muls.

Scheduling: startup DMAs are priority-ordered (x stripe 0 + per-chunk wqkv
first, wo last) so the first proj matmul starts ~15us earlier. Attention is
ACT(exp)-bound late (causal skew) and PE-bound early, so PE filler work is
dripped between the QK and PV of each attention tile from two queues:
"urgent" (proj of stripe st+1, paced to finish within stripe st) and "lazy"
(wo thunks at half-stripe granularity, deferred into the ACT-bound stripes
2-3). Output partials are written bf16 to halve output DMA traffic.
"""

import os
import sys

import numpy as np
import ml_dtypes

BF16 = ml_dtypes.bfloat16

B, S, D, H, KV, HD = 2, 2048, 2048, 32, 8, 64
NCORES = 8
DC = D // 128        # 16 contraction chunks
NQI = S // 256       # 8 q-blocks of 256
KT_TILES = S // 128  # 16 k-blocks of 128
NST = S // 512       # 4 stripes


def _host_prepare(x, wq, wk, wv, wo, freqs, mask):
    """Build per-core device inputs + the mask block schedule."""
    c64 = np.cos(freqs.T).repeat(2, axis=0).astype(np.float64)  # [64, S]
    s64 = np.sin(freqs.T).repeat(2, axis=0).astype(np.float64)
    sgn = np.where(np.arange(HD) % 2 == 0, -1.0, 1.0)[:, None]
    cos_t = np.concatenate([c64, c64], axis=0).astype(BF16)           # [128, S]
    sin_t = np.concatenate([s64 * sgn, s64 * sgn], axis=0).astype(BF16)

    mt_tiles = []
    mt_keys = {}
    sched = []      # per qi: list of (kt, mtile_idx | None)
    for qi in range(NQI):
        lst = []
        for kt in range(KT_TILES):
            blk = mask[qi * 256:(qi + 1) * 256, kt * 128:(kt + 1) * 128]  # [q, k]
            if np.all(blk <= -30.0):
                continue
            if np.all(blk == 0.0):
                lst.append((kt, None))
                continue
            t256 = np.exp(blk.T.astype(np.float64)).astype(BF16)  # [128 k, 256 q]
            tile_np = np.concatenate([t256, t256], axis=1)        # [128, 512]
            key = tile_np.tobytes()
            if key not in mt_keys:
                mt_keys[key] = len(mt_tiles)
                mt_tiles.append(tile_np)
            lst.append((kt, mt_keys[key]))
        if not lst:
            lst = [(kt, None) for kt in range(KT_TILES)]
        sched.append(lst)
    if not mt_tiles:
        mt_tiles.append(np.ones((128, 512), dtype=BF16))
    mt = np.stack(mt_tiles)  # [U, 128, 512]

    # causal fast path: per qi, kt runs 0..2qi+1, all clear except the final
    # kt-pair (2qi: lower-triangular over q-half1, fully visible on q-half2;
    # 2qi+1: fully masked on q-half1, the same lower-triangular on q-half2).
    # One shared [128, 128] exp-triangle tile serves both.
    tri = None
    causal = []
    for qi in range(NQI):
        lst = sched[qi]
        ok = (len(lst) == 2 * qi + 2
              and [kt for kt, _ in lst] == list(range(2 * qi + 2))
              and all(mi is None for _, mi in lst[:-2])
              and lst[-2][1] is not None and lst[-1][1] is not None)
        if ok:
            blk_e = mask[qi * 256:(qi + 1) * 256, 2 * qi * 128:(2 * qi + 1) * 128]
            blk_o = mask[qi * 256:(qi + 1) * 256,
                         (2 * qi + 1) * 128:(2 * qi + 2) * 128]
            te = np.exp(blk_e.T.astype(np.float64))  # [128 k, 256 q]
            to = np.exp(blk_o.T.astype(np.float64))
            t = te[:, 0:128]
            ok = (np.allclose(te[:, 128:256], 1.0)
                  and np.allclose(to[:, 0:128], 0.0)
                  and np.array_equal(to[:, 128:256], t)
                  and (tri is None or np.array_equal(tri, t)))
            if ok:
                tri = t
        causal.append(ok)
    if tri is None:
        tri = np.ones((128, 128), dtype=np.float64)
    mth = np.concatenate([tri, tri], axis=1).astype(BF16)  # [128, 256]

    per_core = []
    for c in range(NCORES):
        b, g = c // 4, c % 4
        xT = np.ascontiguousarray(x[b].T).astype(BF16)
        wqkv = np.concatenate(
            [wq[:, g * 8 * HD:(g + 1) * 8 * HD],
             wk[:, g * 2 * HD:(g + 1) * 2 * HD],
             wv[:, g * 2 * HD:(g + 1) * 2 * HD]], axis=1)
        per_core.append({
            "xT": xT,
            "wqkv": np.ascontiguousarray(wqkv).astype(BF16),
            "wo": np.ascontiguousarray(wo[g * 8 * HD:(g + 1) * 8 * HD, :]).astype(BF16),
            "cos": cos_t,
            "sin": sin_t,
            "mt": mt,
            "mth": mth,
        })
    return per_core, (sched, causal), mt.shape[0]


def _build_program(sched_causal, U):
    sched, causal = sched_causal
    import concourse.bass as bass
    import concourse.mybir as mybir
    import concourse.tile as tile
    from concourse import bacc

    dt = mybir.dt
    bf, f32 = dt.bfloat16, dt.float32
    AF = mybir.ActivationFunctionType

    nc = bacc.Bacc("TRN2", target_bir_lowering=False, debug=False,
                   num_devices=NCORES)

    xT = nc.dram_tensor("xT", [D, S], bf, kind="ExternalInput")
    wqkv = nc.dram_tensor("wqkv", [D, 768], bf, kind="ExternalInput")
    wo = nc.dram_tensor("wo", [512, D], bf, kind="ExternalInput")
    cos = nc.dram_tensor("cos", [128, S], bf, kind="ExternalInput")
    sin = nc.dram_tensor("sin", [128, S], bf, kind="ExternalInput")
    mt = nc.dram_tensor("mt", [U, 128, 512], bf, kind="ExternalInput")
    mth = nc.dram_tensor("mth", [128, 256], bf, kind="ExternalInput")
    out = nc.dram_tensor("out", [S, D], bf, kind="ExternalOutput")

    perm_np = np.zeros((128, 128), dtype=BF16)
    for d in range(128):
        perm_np[d ^ 1, d] = 1
    perm_dram = nc.inline_tensor(np.ascontiguousarray(perm_np), name="perm")
    ident_dram = nc.inline_tensor(np.eye(128, dtype=BF16), name="ident")

    # tiles per stripe (4 pairs x both qi of the stripe)
    TPS = [4 * (len(sched[2 * s]) + len(sched[2 * s + 1])) for s in range(NST)]

    with tile.TileContext(nc) as tc:
        with (
            tc.tile_pool(name="const", bufs=1) as cp,
            tc.tile_pool(name="xt", bufs=2) as xp,
            tc.tile_pool(name="raw", bufs=3) as rawp,
            tc.tile_pool(name="rtmp", bufs=2) as rtp,
            tc.tile_pool(name="pt", bufs=6) as ptp,
            tc.tile_pool(name="ot", bufs=4) as otp,
            tc.tile_pool(name="bc", bufs=2) as bcp,
            tc.tile_pool(name="ri", bufs=2) as rip,
            tc.tile_pool(name="wsb", bufs=3) as wsp,
            tc.tile_pool(name="ps_p", bufs=1, space="PSUM") as pp,
            tc.tile_pool(name="ps_s", bufs=2, space="PSUM") as pss,
            tc.tile_pool(name="ps_o", bufs=1, space="PSUM") as pso,
            tc.tile_pool(name="ps_w", bufs=2, space="PSUM") as psw,
        ):
            # ---- priority-ordered startup DMAs ----
            # ordered by first-use time: x stripe 0 + K/V weight chunks gate
            # the first proj matmuls; wo is only consumed from stripe 1 on.
            xbigs = {}
            xr = xT.ap().rearrange("(c p) s -> p c s", p=128)
            xbig0 = xp.tile([128, DC, 512], bf, tag="x")
            xbigs[0] = xbig0
            wqkv_sb = cp.tile([128, DC, 768], bf)
            wr = wqkv.ap().rearrange("(c p) m -> p c m", p=128)

            def dma_wqkv(m):
                nc.sync.dma_start(wqkv_sb[:, :, m * 128:(m + 1) * 128],
                                  wr[:, :, m * 128:(m + 1) * 128])

            nc.sync.dma_start(xbig0[:, 0:4, :], xr[:, 0:4, 0:512])
            dma_wqkv(4)
            for sub in range(1, 4):
                nc.sync.dma_start(xbig0[:, 4 * sub:4 * sub + 4, :],
                                  xr[:, 4 * sub:4 * sub + 4, 0:512])
            dma_wqkv(5)
            perm_sb = cp.tile([128, 128], bf)
            nc.sync.dma_start(perm_sb[:], perm_dram.ap())
            ident_sb = cp.tile([128, 128], bf)
            nc.sync.dma_start(ident_sb[:], ident_dram.ap())
            cos_sb = cp.tile([128, S], bf)
            nc.sync.dma_start(cos_sb[:], cos.ap())
            sin_sb = cp.tile([128, S], bf)
            nc.sync.dma_start(sin_sb[:], sin.ap())
            mt_sb = cp.tile([128, U, 512], bf)
            nc.sync.dma_start(mt_sb[:], mt.ap().rearrange("u p q -> p u q"))
            mth_sb = cp.tile([128, 2, 128], bf)
            nc.sync.dma_start(mth_sb[:], mth.ap().rearrange("p (h q) -> p h q", h=2))
            for m in (0, 1, 2, 3):
                dma_wqkv(m)
            # wo: first consumed during stripe 1+ attention
            wo_sb = cp.tile([128, 4, D], bf)
            nc.sync.dma_start(wo_sb[:], wo.ap().rearrange("(g p) n -> p g n", p=128))

            qt_sb = cp.tile([64, 4, 2, S], bf)  # [dh, pair, head-in-pair, s]
            kt_sb = cp.tile([64, 2, S], bf)     # [dh, kv, s]
            vone_sb = cp.tile([128, KT_TILES, 130], bf)
            nc.vector.memset(vone_sb[:, :, 64:65], 1.0)
            nc.vector.memset(vone_sb[:, :, 129:130], 1.0)

            ot_ts = {}

            # ---------- thunk builders ----------
            def proj_thunks(st, with_dma=True):
                """DMA + proj chunks (K,V first) + rope/V-transpose thunks.

                Stripe 0 (the serial head) allocates its psum chunks from the
                score ring (3 bufs, free before attention starts) so chunks
                double-buffer; later stripes use the single proj bank. The
                psum->bf16 raw cast runs on ACT for stripes whose host stripe
                is PE-bound (proj(st) runs during stripe st-1), keeping the
                DVE queue clear; proj(3) runs during the ACT-bound stripe 2,
                so it stays on DVE."""
                s0 = st * 512
                ths = []
                state = {}
                use_act_cast = True

                def t_dma():
                    xbig = xp.tile([128, DC, 512], bf, tag="x")
                    xbigs[st] = xbig
                    for sub in range(4):
                        nc.sync.dma_start(xbig[:, 4 * sub:4 * sub + 4, :],
                                          xr[:, 4 * sub:4 * sub + 4, s0:s0 + 512])
                if with_dma:
                    ths.append(t_dma)

                def mk_mm(m, lo, hi, first, last):
                    def t():
                        if first:
                            if st == 0:
                                big = pss.tile([128, 1024], f32, tag="sc",
                                               name="projps0")
                                state[m] = big[:, 0:512]
                            else:
                                state[m] = pp.tile([128, 512], f32, tag="proj",
                                                   name="projps")
                        ps = state[m]
                        xbig = xbigs[st]
                        for dc in range(lo, hi):
                            nc.tensor.matmul(
                                ps[:], lhsT=wqkv_sb[:, dc, m * 128:(m + 1) * 128],
                                rhs=xbig[:, dc, :],
                                start=(dc == 0), stop=(dc == DC - 1))
                        if last:
                            raw = rawp.tile([128, 512], bf, tag="raw")
                            state[(m, "raw")] = raw
                            if use_act_cast:
                                nc.scalar.activation(raw[:], ps[:], AF.Copy)
                            else:
                                nc.vector.tensor_copy(raw[:], ps[:])
                    return t

                def mk_rope(m):
                    def t():
                        raw = state[(m, 'raw')]
                        sw = psw.tile([128, 512], f32, tag="wo", name="sw")
                        nc.tensor.matmul(sw[:], lhsT=perm_sb[:], rhs=raw[:],
                                         start=True, stop=True)
                        tsin = rtp.tile([128, 512], bf, tag="tsin")
                        nc.vector.tensor_mul(tsin[:], sw[:], sin_sb[:, s0:s0 + 512])
                        tcos = rtp.tile([128, 512], bf, tag="tcos")
                        nc.vector.tensor_mul(tcos[:], raw[:], cos_sb[:, s0:s0 + 512])
                        rot = rtp.tile([128, 512], bf, tag="rot")
                        nc.vector.tensor_add(rot[:], tsin[:], tcos[:])
                        if m < 4:
                            nc.vector.tensor_copy(qt_sb[:, m, 0, s0:s0 + 512],
                                                  rot[0:64, :])
                            nc.vector.tensor_copy(qt_sb[:, m, 1, s0:s0 + 512],
                                                  rot[64:128, :])
                        else:
                            nc.vector.tensor_copy(kt_sb[:, 0, s0:s0 + 512],
                                                  rot[0:64, :])
                            nc.vector.tensor_copy(kt_sb[:, 1, s0:s0 + 512],
                                                  rot[64:128, :])
                    return t

                def mk_vt(j):
                    def t():
                        raw = state[(5, 'raw')]
                        mv = psw.tile([128, 512], f32, tag="wo", name="mv")
                        vt = mv[:, 0:64].bitcast(bf)
                        nc.tensor.transpose(vt, raw[:, j * 128:(j + 1) * 128],
                                            ident_sb[:])
                        kt_idx = 4 * st + j
                        nc.vector.tensor_copy(vone_sb[:, kt_idx, 0:64],
                                              vt[:, 0:64])
                        nc.vector.tensor_copy(vone_sb[:, kt_idx, 65:129],
                                              vt[:, 64:128])
                    return t

                chunks = {}
                for m in (4, 5, 0, 1, 2, 3):
                    sub = []
                    for q in range(4):
                        sub.append(mk_mm(m, 4 * q, 4 * q + 4, q == 0, q == 3))
                    if m == 5:
                        for j in range(4):
                            sub.append(mk_vt(j))
                    else:
                        sub.append(mk_rope(m))
                    chunks[m] = sub
                    ths += sub
                ths_by_chunk = chunks
                return ths, ths_by_chunk

            def wo_half_thunks(st, half):
                """wo for q rows [st*512 + half*256, +256): ready once both
                qi=2*st+half normalizations of all 4 pairs are done."""
                s0 = st * 512
                ths = []

                def mk(j, nb):
                    def t():
                        ot_t = ot_ts[st]
                        wp = psw.tile([128, 512], f32, tag="wo")
                        for pr in range(4):
                            nc.tensor.matmul(
                                wp[:], lhsT=ot_t[:, pr, j * 128:(j + 1) * 128],
                                rhs=wo_sb[:, pr, nb * 512:(nb + 1) * 512],
                                start=(pr == 0), stop=(pr == 3))
                        wsb = wsp.tile([128, 512], bf, tag="wsb")
                        nc.vector.tensor_copy(wsb[:], wp[:])
                        nc.sync.dma_start(
                            out.ap()[s0 + j * 128:s0 + (j + 1) * 128,
                                     nb * 512:(nb + 1) * 512],
                            wsb[:])
                    return t

                for j in (2 * half, 2 * half + 1):
                    for nb in range(4):
                        ths.append(mk(j, nb))
                return ths

            # ---------- dripper: paced filler queues ----------
            class Dripper:
                def __init__(self):
                    self.urgent = []   # proj(st+1): drain within stripe st
                    self.lazy = []     # wo halves: drain during stripes >= 2
                    self.ucred = 0.0
                    self.lcred = 0.0

                def tick(self, tiles_left_stripe, tiles_left_total, lazy_on):
                    if self.urgent:
                        self.ucred += len(self.urgent) / max(1, tiles_left_stripe)
                        while self.ucred >= 1.0 - 1e-9 and self.urgent:
                            self.urgent.pop(0)()
                            self.ucred -= 1.0
                    else:
                        self.ucred = 0.0
                    if lazy_on and self.lazy:
                        self.lcred += min(1.0, len(self.lazy)
                                          / max(1, tiles_left_total))
                        while self.lcred >= 1.0 - 1e-9 and self.lazy:
                            self.lazy.pop(0)()
                            self.lcred -= 1.0

                def drain(self):
                    for t in self.urgent:
                        t()
                    for t in self.lazy:
                        t()
                    self.urgent, self.lazy = [], []

            dr = Dripper()
            tiles_done = [0]
            TOT = sum(TPS)

            def emit_attn_pair(st, qi, pair, done_in_stripe, tick=True):
                s0 = st * 512
                q0 = qi * 256
                kv = pair // 2
                kts = sched[qi]

                def dripn(n):
                    nonlocal done_in_stripe
                    for _ in range(n):
                        if tick:
                            dr.tick(TPS[st] - done_in_stripe,
                                    TOT - tiles_done[0], st >= 2)
                        tiles_done[0] += 1
                        done_in_stripe += 1

                if causal[qi]:
                    # kt-pair path: two score matmuls into one 2-bank psum
                    # tile, a single merged exp, triangular mask on the
                    # diagonal kt-pair only (its odd tile is computed at
                    # half width; the fully-masked q-half1 is skipped).
                    # The diagonal pair runs FIRST so the mask-mul DVE hop
                    # is out of the pair-end chain that frees the psum bank.
                    otps = pso.tile([128, 2, 256], f32, tag="otp")
                    npair = len(kts) // 2
                    for seq, j in enumerate([npair - 1] + list(range(npair - 1))):
                        kt0, kt1 = 2 * j, 2 * j + 1
                        diag = (j == npair - 1)
                        first = (seq == 0)
                        last = (seq == npair - 1)
                        sp = pss.tile([128, 1024], f32, tag="sc")
                        nc.tensor.matmul(
                            sp[:, 0:512],
                            lhsT=kt_sb[:, kv, kt0 * 128:(kt0 + 1) * 128],
                            rhs=qt_sb[:, pair, :, q0:q0 + 256],
                            start=True, stop=True)
                        if diag:
                            nc.tensor.matmul(
                                sp[:, 512:768],
                                lhsT=kt_sb[:, kv, kt1 * 128:(kt1 + 1) * 128],
                                rhs=qt_sb[:, pair, :, q0 + 128:q0 + 256],
                                start=True, stop=True)
                        else:
                            nc.tensor.matmul(
                                sp[:, 512:1024],
                                lhsT=kt_sb[:, kv, kt1 * 128:(kt1 + 1) * 128],
                                rhs=qt_sb[:, pair, :, q0:q0 + 256],
                                start=True, stop=True)
                        pt = ptp.tile([128, 1024], bf, tag="pt")
                        ncols = 768 if diag else 1024
                        nc.scalar.activation(pt[:, 0:ncols], sp[:, 0:ncols],
                                             AF.Exp, scale=1.0 / np.sqrt(HD))
                        if diag:
                            pe = pt[:, 0:512].rearrange("p (h q) -> p h q", h=2)
                            nc.vector.tensor_mul(pe[:, :, 0:128],
                                                 pe[:, :, 0:128], mth_sb[:])
                            po = pt[:, 512:768].rearrange(
                                "p (h q) -> p h q", h=2)
                            nc.vector.tensor_mul(po[:], po[:], mth_sb[:])
                        dripn(2)
                        nc.tensor.matmul(
                            otps[0:65, :, :],
                            lhsT=vone_sb[:, kt0, 65 * kv:65 * kv + 65],
                            rhs=pt[:, 0:512],
                            start=first, stop=False,
                            skip_group_check=True)
                        if diag:
                            for h in range(2):
                                nc.tensor.matmul(
                                    otps[0:65, h, 128:256],
                                    lhsT=vone_sb[:, kt1, 65 * kv:65 * kv + 65],
                                    rhs=pt[:, 512 + h * 128:640 + h * 128],
                                    start=False, stop=(last and h == 1),
                                    skip_group_check=True)
                        else:
                            nc.tensor.matmul(
                                otps[0:65, :, :],
                                lhsT=vone_sb[:, kt1, 65 * kv:65 * kv + 65],
                                rhs=pt[:, 512:1024],
                                start=False, stop=last,
                                skip_group_check=True)
                    otps = otps[:, :, :].rearrange("p h q -> p (h q)")
                else:
                    otps = pso.tile([128, 512], f32, tag="otp")
                    for idx, (kt, mi) in enumerate(kts):
                        spb = pss.tile([128, 1024], f32, tag="sc")
                        sp = spb[:, 0:512]
                        nc.tensor.matmul(
                            sp,
                            lhsT=kt_sb[:, kv, kt * 128:(kt + 1) * 128],
                            rhs=qt_sb[:, pair, :, q0:q0 + 256],
                            start=True, stop=True)
                        pt = ptp.tile([128, 1024], bf, tag="pt")
                        nc.scalar.activation(pt[:, 0:512], sp, AF.Exp,
                                             scale=1.0 / np.sqrt(HD))
                        if mi is not None:
                            nc.vector.tensor_mul(pt[:, 0:512], pt[:, 0:512],
                                                 mt_sb[:, mi, :])
                        dripn(1)
                        nc.tensor.matmul(
                            otps[0:65, :],
                            lhsT=vone_sb[:, kt, 65 * kv:65 * kv + 65],
                            rhs=pt[:, 0:512],
                            start=(idx == 0), stop=(idx == len(kts) - 1))
                # normalization (off the PE critical path); a single [0:65]
                # copy frees the psum bank in one DVE op
                orw = bcp.tile([65, 512], bf, tag="orw")
                nc.vector.tensor_copy(orw[:], otps[0:65, :])
                rs1 = rip.tile([1, 512], f32, tag="rs1")
                nc.vector.tensor_copy(rs1[:], orw[64:65, :])
                ri = rip.tile([1, 512], f32, tag="ri")
                nc.vector.reciprocal_approx_fast(ri[:], rs1[:])
                rib = rip.tile([1, 512], bf, tag="rib")
                nc.vector.tensor_copy(rib[:], ri[:])
                bcsf = bcp.tile([64, 512], bf, tag="bc")
                nc.gpsimd.partition_broadcast(bcsf[:], rib[:])
                qo = q0 - s0
                ot_t = ot_ts[st]
                nc.vector.tensor_mul(ot_t[0:64, pair, qo:qo + 256],
                                     orw[0:64, 0:256], bcsf[:, 0:256])
                nc.vector.tensor_mul(ot_t[64:128, pair, qo:qo + 256],
                                     orw[0:64, 256:512], bcsf[:, 256:512])
                return done_in_stripe

            # ---------- emission ----------
            # Stripe-0 head: K/V/Q0 proj chunks first, then qi=0 attention
            # pairs woven between the remaining q-pair proj chunks so exp
            # streaming starts while proj still owns the PE.
            _, p0 = proj_thunks(0, with_dma=False)
            ot_ts[0] = otp.tile([128, 4, 512], bf, tag="ot_t", name="ot_t0")
            for t in p0[4] + p0[5] + p0[0]:
                t()
            n0 = 0
            for pair in range(4):
                if pair + 1 <= 3:
                    for t in p0[pair + 1]:
                        t()
                n0 = emit_attn_pair(0, 0, pair, n0, tick=False)
            dr.lazy += wo_half_thunks(0, 0)
            for st in range(NST):
                if st + 1 < NST:
                    ths, _ = proj_thunks(st + 1)
                    dr.urgent += ths
                if st > 0:
                    ot_ts[st] = otp.tile([128, 4, 512], bf, tag="ot_t",
                                         name="ot_t")
                    n = 0
                    for pair in range(4):
                        n = emit_attn_pair(st, 2 * st, pair, n)
                    dr.lazy += wo_half_thunks(st, 0)
                else:
                    n = n0
                for pair in range(4):
                    n = emit_attn_pair(st, 2 * st + 1, pair, n)
                dr.lazy += wo_half_thunks(st, 1)
            dr.drain()
    nc.compile()
    return nc


def kernel(x, wq, wk, wv, wo, freqs, mask, start_pos):
    sys.path.insert(0, "/opt/trn_rl_repo")
    from concourse.bass_utils import run_bass_kernel_spmd

    x = np.asarray(x, dtype=np.float32)
    per_core, sched, U = _host_prepare(
        x, np.asarray(wq, np.float32), np.asarray(wk, np.float32),
        np.asarray(wv, np.float32), np.asarray(wo, np.float32),
        np.asarray(freqs, np.float32), np.asarray(mask, np.float32))

    nc = _build_program(sched, U)

    trace = bool(int(os.environ.get("BASSKERNEL_TRACE", "0")))
    if trace and "antenv.axon_hooks" not in sys.modules:
        try:
            import types

            if "/root/.axon_site" not in sys.path:
                sys.path.insert(0, "/root/.axon_site")
            from trn_agent_boot.trn_boot import _ntff_profile_via_ctypes

            _hook = _ntff_profile_via_ctypes("/opt/axon/libaxon_pjrt.so")
            _mod = types.ModuleType("antenv.axon_hooks")
            _mod.get_axon_ntff_profile_hook = lambda: _hook
            _mod.set_axon_ntff_profile_hook = lambda h: None
            sys.modules["antenv.axon_hooks"] = _mod
        except Exception:
            trace = False
    res = run_bass_kernel_spmd(nc, per_core, core_ids=list(range(NCORES)),
                               trace=trace)
    if trace:
        kernel._last_exec_time_ns = res.exec_time_ns
        kernel._last_profile = res.profile_json
    full = np.empty((B, S, D), np.float32)
    for b in range(B):
        acc = res.results[4 * b]["out"].astype(np.float64)
        for g in range(1, 4):
            acc += res.results[4 * b + g]["out"].astype(np.float64)
        full[b] = acc.astype(np.float32)
    return full


# revision 18
# speedup vs baseline: 1.1959x; 1.1959x over previous
"""GQA prefill attention (B=2, S=2048, D=2048, H=32, KV=8, HD=64) on 8 trn2 cores.

Sharding: batch x kv-pair. Core c = (b = c//4, g = c%4) owns batch b,
q-heads [8g, 8g+8) and kv-heads {2g, 2g+1}; computes its partial of
out_b = attn_out @ wo_chunk; host sums the 4 partials per batch.

Per 512-seq stripe: projections (6 psum chunks: 4 q-head-pairs, K, V) with
RoPE via pair-swap permutation matmul; V transposed to [s, dh] on PE.
Attention qi-outer / pair-inner per 256-q block: both heads of a pair share
the kv head, so one K=64 score matmul with rhs=[Q_even|Q_odd] fills one psum
bank [128k, 256q_e | 256q_o]; exp on ACT; partially masked blocks multiplied
by exp(mask^T) tiles; one PV matmul lhsT=[V|1] rhs=P gives OT[dh|rowsum, 512]
accumulated over k-blocks. Normalization: OT+rowsum copied to SBUF (frees
the bank), approx-reciprocal on DVE, partition-broadcast on GpSimd, two
DVE muls.

Scheduling: startup DMAs are priority-ordered (x stripe 0 + per-chunk wqkv
first, wo last) so the first proj matmul starts ~15us earlier. Attention is
ACT(exp)-bound late (causal skew) and PE-bound early, so PE filler work is
dripped between the QK and PV of each attention tile from two queues:
"urgent" (proj of stripe st+1, paced to finish within stripe st) and "lazy"
(wo thunks at half-stripe granularity, deferred into the ACT-bound stripes
2-3). Output partials are written bf16 to halve output DMA traffic.
"""

import os
import sys

import numpy as np
import ml_dtypes

BF16 = ml_dtypes.bfloat16

B, S, D, H, KV, HD = 2, 2048, 2048, 32, 8, 64
NCORES = 8
DC = D // 128        # 16 contraction chunks
NQI = S // 256       # 8 q-blocks of 256
KT_TILES = S // 128  # 16 k-blocks of 128
NST = S // 512       # 4 stripes


def _host_prepare(x, wq, wk, wv, wo, freqs, mask):
    """Build per-core device inputs + the mask block schedule."""
    c64 = np.cos(freqs.T).repeat(2, axis=0).astype(np.float64)  # [64, S]
    s64 = np.sin(freqs.T).repeat(2, axis=0).astype(np.float64)
    sgn = np.where(np.arange(HD) % 2 == 0, -1.0, 1.0)[:, None]
    cos_t = np.concatenate([c64, c64], axis=0).astype(BF16)           # [128, S]
    sin_t = np.concatenate([s64 * sgn, s64 * sgn], axis=0).astype(BF16)

    mt_tiles = []
    mt_keys = {}
    sched = []      # per qi: list of (kt, mtile_idx | None)
    for qi in range(NQI):
        lst = []
        for kt in range(KT_TILES):
            blk = mask[qi * 256:(qi + 1) * 256, kt * 128:(kt + 1) * 128]  # [q, k]
            if np.all(blk <= -30.0):
                continue
            if np.all(blk == 0.0):
                lst.append((kt, None))
                continue
            t256 = np.exp(blk.T.astype(np.float64)).astype(BF16)  # [128 k, 256 q]
            tile_np = np.concatenate([t256, t256], axis=1)        # [128, 512]
            key = tile_np.tobytes()
            if key not in mt_keys:
                mt_keys[key] = len(mt_tiles)
                mt_tiles.append(tile_np)
            lst.append((kt, mt_keys[key]))
        if not lst:
            lst = [(kt, None) for kt in range(KT_TILES)]
        sched.append(lst)
    if not mt_tiles:
        mt_tiles.append(np.ones((128, 512), dtype=BF16))
    mt = np.stack(mt_tiles)  # [U, 128, 512]

    # causal fast path: per qi, kt runs 0..2qi+1, all clear except the final
    # kt-pair (2qi: lower-triangular over q-half1, fully visible on q-half2;
    # 2qi+1: fully masked on q-half1, the same lower-triangular on q-half2).
    # One shared [128, 128] exp-triangle tile serves both.
    tri = None
    causal = []
    for qi in range(NQI):
        lst = sched[qi]
        ok = (len(lst) == 2 * qi + 2
              and [kt for kt, _ in lst] == list(range(2 * qi + 2))
              and all(mi is None for _, mi in lst[:-2])
              and lst[-2][1] is not None and lst[-1][1] is not None)
        if ok:
            blk_e = mask[qi * 256:(qi + 1) * 256, 2 * qi * 128:(2 * qi + 1) * 128]
            blk_o = mask[qi * 256:(qi + 1) * 256,
                         (2 * qi + 1) * 128:(2 * qi + 2) * 128]
            te = np.exp(blk_e.T.astype(np.float64))  # [128 k, 256 q]
            to = np.exp(blk_o.T.astype(np.float64))
            t = te[:, 0:128]
            ok = (np.allclose(te[:, 128:256], 1.0)
                  and np.allclose(to[:, 0:128], 0.0)
                  and np.array_equal(to[:, 128:256], t)
                  and (tri is None or np.array_equal(tri, t)))
            if ok:
                tri = t
        causal.append(ok)
    if tri is None:
        tri = np.ones((128, 128), dtype=np.float64)
    mth = np.concatenate([tri, tri], axis=1).astype(BF16)  # [128, 256]

    per_core = []
    for c in range(NCORES):
        b, g = c // 4, c % 4
        xT = np.ascontiguousarray(x[b].T).astype(BF16)
        wqkv = np.concatenate(
            [wq[:, g * 8 * HD:(g + 1) * 8 * HD],
             wk[:, g * 2 * HD:(g + 1) * 2 * HD],
             wv[:, g * 2 * HD:(g + 1) * 2 * HD]], axis=1)
        per_core.append({
            "xT": xT,
            "wqkv": np.ascontiguousarray(wqkv).astype(BF16),
            "wo": np.ascontiguousarray(wo[g * 8 * HD:(g + 1) * 8 * HD, :]).astype(BF16),
            "cos": cos_t,
            "sin": sin_t,
            "mt": mt,
            "mth": mth,
        })
    return per_core, (sched, causal), mt.shape[0]


def _build_program(sched_causal, U):
    sched, causal = sched_causal
    import concourse.bass as bass
    import concourse.mybir as mybir
    import concourse.tile as tile
    from concourse import bacc

    dt = mybir.dt
    bf, f32 = dt.bfloat16, dt.float32
    AF = mybir.ActivationFunctionType

    nc = bacc.Bacc("TRN2", target_bir_lowering=False, debug=False,
                   num_devices=NCORES)

    xT = nc.dram_tensor("xT", [D, S], bf, kind="ExternalInput")
    wqkv = nc.dram_tensor("wqkv", [D, 768], bf, kind="ExternalInput")
    wo = nc.dram_tensor("wo", [512, D], bf, kind="ExternalInput")
    cos = nc.dram_tensor("cos", [128, S], bf, kind="ExternalInput")
    sin = nc.dram_tensor("sin", [128, S], bf, kind="ExternalInput")
    mt = nc.dram_tensor("mt", [U, 128, 512], bf, kind="ExternalInput")
    mth = nc.dram_tensor("mth", [128, 256], bf, kind="ExternalInput")
    out = nc.dram_tensor("out", [S, D], bf, kind="ExternalOutput")

    perm_np = np.zeros((128, 128), dtype=BF16)
    for d in range(128):
        perm_np[d ^ 1, d] = 1
    perm_dram = nc.inline_tensor(np.ascontiguousarray(perm_np), name="perm")
    ident_dram = nc.inline_tensor(np.eye(128, dtype=BF16), name="ident")

    # tiles per stripe (4 pairs x both qi of the stripe)
    TPS = [4 * (len(sched[2 * s]) + len(sched[2 * s + 1])) for s in range(NST)]

    with tile.TileContext(nc) as tc:
        with (
            tc.tile_pool(name="const", bufs=1) as cp,
            tc.tile_pool(name="xt", bufs=2) as xp,
            tc.tile_pool(name="raw", bufs=3) as rawp,
            tc.tile_pool(name="rtmp", bufs=2) as rtp,
            tc.tile_pool(name="pt", bufs=6) as ptp,
            tc.tile_pool(name="ot", bufs=4) as otp,
            tc.tile_pool(name="bc", bufs=2) as bcp,
            tc.tile_pool(name="ri", bufs=2) as rip,
            tc.tile_pool(name="wsb", bufs=3) as wsp,
            tc.tile_pool(name="ps_p", bufs=1, space="PSUM") as pp,
            tc.tile_pool(name="ps_s", bufs=2, space="PSUM") as pss,
            tc.tile_pool(name="ps_o", bufs=1, space="PSUM") as pso,
            tc.tile_pool(name="ps_w", bufs=2, space="PSUM") as psw,
        ):
            # ---- priority-ordered startup DMAs ----
            # ordered by first-use time: x stripe 0 + K/V weight chunks gate
            # the first proj matmuls; wo is only consumed from stripe 1 on.
            xbigs = {}
            xr = xT.ap().rearrange("(c p) s -> p c s", p=128)
            xbig0 = xp.tile([128, DC, 512], bf, tag="x")
            xbigs[0] = xbig0
            wqkv_sb = cp.tile([128, DC, 768], bf)
            wr = wqkv.ap().rearrange("(c p) m -> p c m", p=128)

            def dma_wqkv(m):
                nc.sync.dma_start(wqkv_sb[:, :, m * 128:(m + 1) * 128],
                                  wr[:, :, m * 128:(m + 1) * 128])

            nc.sync.dma_start(xbig0[:, 0:4, :], xr[:, 0:4, 0:512])
            dma_wqkv(4)
            for sub in range(1, 4):
                nc.sync.dma_start(xbig0[:, 4 * sub:4 * sub + 4, :],
                                  xr[:, 4 * sub:4 * sub + 4, 0:512])
            dma_wqkv(5)
            perm_sb = cp.tile([128, 128], bf)
            nc.sync.dma_start(perm_sb[:], perm_dram.ap())
            ident_sb = cp.tile([128, 128], bf)
            nc.sync.dma_start(ident_sb[:], ident_dram.ap())
            cos_sb = cp.tile([128, S], bf)
            nc.sync.dma_start(cos_sb[:], cos.ap())
            sin_sb = cp.tile([128, S], bf)
            nc.sync.dma_start(sin_sb[:], sin.ap())
            mt_sb = cp.tile([128, U, 512], bf)
            nc.sync.dma_start(mt_sb[:], mt.ap().rearrange("u p q -> p u q"))
            mth_sb = cp.tile([128, 2, 128], bf)
            nc.sync.dma_start(mth_sb[:], mth.ap().rearrange("p (h q) -> p h q", h=2))
            for m in (0, 1, 2, 3):
                dma_wqkv(m)
            # wo: first consumed during stripe 1+ attention
            wo_sb = cp.tile([128, 4, D], bf)
            nc.sync.dma_start(wo_sb[:], wo.ap().rearrange("(g p) n -> p g n", p=128))

            qt_sb = cp.tile([64, 4, 2, S], bf)  # [dh, pair, head-in-pair, s]
            kt_sb = cp.tile([64, 2, S], bf)     # [dh, kv, s]
            vone_sb = cp.tile([128, KT_TILES, 130], bf)
            nc.vector.memset(vone_sb[:, :, 64:65], 1.0)
            nc.vector.memset(vone_sb[:, :, 129:130], 1.0)

            ot_ts = {}

            # ---------- thunk builders ----------
            def proj_thunks(st, with_dma=True):
                """DMA + proj chunks (K,V first) + rope/V-transpose thunks.

                Stripe 0 (the serial head) allocates its psum chunks from the
                score ring (3 bufs, free before attention starts) so chunks
                double-buffer; later stripes use the single proj bank. The
                psum->bf16 raw cast runs on ACT for stripes whose host stripe
                is PE-bound (proj(st) runs during stripe st-1), keeping the
                DVE queue clear; proj(3) runs during the ACT-bound stripe 2,
                so it stays on DVE."""
                s0 = st * 512
                ths = []
                state = {}
                use_act_cast = True

                def t_dma():
                    xbig = xp.tile([128, DC, 512], bf, tag="x")
                    xbigs[st] = xbig
                    for sub in range(4):
                        nc.sync.dma_start(xbig[:, 4 * sub:4 * sub + 4, :],
                                          xr[:, 4 * sub:4 * sub + 4, s0:s0 + 512])
                if with_dma:
                    ths.append(t_dma)

                def mk_mm(m, lo, hi, first, last):
                    def t():
                        if first:
                            if st == 0:
                                big = pss.tile([128, 1024], f32, tag="sc",
                                               name="projps0")
                                state[m] = big[:, 0:512]
                            else:
                                state[m] = pp.tile([128, 512], f32, tag="proj",
                                                   name="projps")
                        ps = state[m]
                        xbig = xbigs[st]
                        for dc in range(lo, hi):
                            nc.tensor.matmul(
                                ps[:], lhsT=wqkv_sb[:, dc, m * 128:(m + 1) * 128],
                                rhs=xbig[:, dc, :],
                                start=(dc == 0), stop=(dc == DC - 1))
                        if last:
                            raw = rawp.tile([128, 512], bf, tag="raw")
                            state[(m, "raw")] = raw
                            if use_act_cast:
                                nc.scalar.activation(raw[:], ps[:], AF.Copy)
                            else:
                                nc.vector.tensor_copy(raw[:], ps[:])
                    return t

                def mk_rope(m):
                    def t():
                        raw = state[(m, 'raw')]
                        sw = psw.tile([128, 512], f32, tag="wo", name="sw")
                        nc.tensor.matmul(sw[:], lhsT=perm_sb[:], rhs=raw[:],
                                         start=True, stop=True)
                        tsin = rtp.tile([128, 512], bf, tag="tsin")
                        nc.vector.tensor_mul(tsin[:], sw[:], sin_sb[:, s0:s0 + 512])
                        tcos = rtp.tile([128, 512], bf, tag="tcos")
                        nc.vector.tensor_mul(tcos[:], raw[:], cos_sb[:, s0:s0 + 512])
                        rot = rtp.tile([128, 512], bf, tag="rot")
                        nc.vector.tensor_add(rot[:], tsin[:], tcos[:])
                        if m < 4:
                            nc.vector.tensor_copy(qt_sb[:, m, 0, s0:s0 + 512],
                                                  rot[0:64, :])
                            nc.vector.tensor_copy(qt_sb[:, m, 1, s0:s0 + 512],
                                                  rot[64:128, :])
                        else:
                            nc.vector.tensor_copy(kt_sb[:, 0, s0:s0 + 512],
                                                  rot[0:64, :])
                            nc.vector.tensor_copy(kt_sb[:, 1, s0:s0 + 512],
                                                  rot[64:128, :])
                    return t

                def mk_vt(j):
                    def t():
                        raw = state[(5, 'raw')]
                        mv = psw.tile([128, 512], f32, tag="wo", name="mv")
                        vt = mv[:, 0:64].bitcast(bf)
                        nc.tensor.transpose(vt, raw[:, j * 128:(j + 1) * 128],
                                            ident_sb[:])
                        kt_idx = 4 * st + j
                        nc.vector.tensor_copy(vone_sb[:, kt_idx, 0:64],
                                              vt[:, 0:64])
                        nc.vector.tensor_copy(vone_sb[:, kt_idx, 65:129],
                                              vt[:, 64:128])
                    return t

                chunks = {}
                for m in (4, 5, 0, 1, 2, 3):
                    sub = []
                    for q in range(4):
                        sub.append(mk_mm(m, 4 * q, 4 * q + 4, q == 0, q == 3))
                    if m == 5:
                        for j in range(4):
                            sub.append(mk_vt(j))
                    else:
                        sub.append(mk_rope(m))
                    chunks[m] = sub
                    ths += sub
                ths_by_chunk = chunks
                return ths, ths_by_chunk

            def wo_half_thunks(st, half):
                """wo for q rows [st*512 + half*256, +256): ready once both
                qi=2*st+half normalizations of all 4 pairs are done."""
                s0 = st * 512
                ths = []

                def mk(j, nb):
                    def t():
                        ot_t = ot_ts[st]
                        wp = psw.tile([128, 512], f32, tag="wo")
                        for pr in range(4):
                            nc.tensor.matmul(
                                wp[:], lhsT=ot_t[:, pr, j * 128:(j + 1) * 128],
                                rhs=wo_sb[:, pr, nb * 512:(nb + 1) * 512],
                                start=(pr == 0), stop=(pr == 3))
                        wsb = wsp.tile([128, 512], bf, tag="wsb")
                        nc.vector.tensor_copy(wsb[:], wp[:])
                        nc.sync.dma_start(
                            out.ap()[s0 + j * 128:s0 + (j + 1) * 128,
                                     nb * 512:(nb + 1) * 512],
                            wsb[:])
                    return t

                for j in (2 * half, 2 * half + 1):
                    for nb in range(4):
                        ths.append(mk(j, nb))
                return ths

            # ---------- dripper: paced filler queues ----------
            class Dripper:
                def __init__(self):
                    self.urgent = []   # proj(st+1): drain within stripe st
                    self.lazy = []     # wo halves: drain during stripes >= 2
                    self.ucred = 0.0
                    self.lcred = 0.0

                def tick(self, tiles_left_stripe, tiles_left_total, lazy_on):
                    if self.urgent:
                        self.ucred += len(self.urgent) / max(1, tiles_left_stripe)
                        while self.ucred >= 1.0 - 1e-9 and self.urgent:
                            self.urgent.pop(0)()
                            self.ucred -= 1.0
                    else:
                        self.ucred = 0.0
                    if lazy_on and self.lazy:
                        self.lcred += min(1.0, len(self.lazy)
                                          / max(1, tiles_left_total))
                        while self.lcred >= 1.0 - 1e-9 and self.lazy:
                            self.lazy.pop(0)()
                            self.lcred -= 1.0

                def drain(self):
                    for t in self.urgent:
                        t()
                    for t in self.lazy:
                        t()
                    self.urgent, self.lazy = [], []

            dr = Dripper()
            tiles_done = [0]
            TOT = sum(TPS)

            def emit_attn_pair(st, qi, pair, done_in_stripe, tick=True):
                s0 = st * 512
                q0 = qi * 256
                kv = pair // 2
                kts = sched[qi]

                def dripn(n):
                    nonlocal done_in_stripe
                    for _ in range(n):
                        if tick:
                            dr.tick(TPS[st] - done_in_stripe,
                                    TOT - tiles_done[0], st >= 2)
                        tiles_done[0] += 1
                        done_in_stripe += 1

                if causal[qi]:
                    # kt-pair path: two score matmuls into one 2-bank psum
                    # tile, a single merged exp, triangular mask on the
                    # diagonal kt-pair only (its odd tile is computed at
                    # half width; the fully-masked q-half1 is skipped).
                    otps = pso.tile([128, 2, 256], f32, tag="otp")
                    npair = len(kts) // 2
                    for j in range(npair):
                        kt0, kt1 = 2 * j, 2 * j + 1
                        diag = (j == npair - 1)
                        first = (j == 0)
                        last = (j == npair - 1)
                        sp = pss.tile([128, 1024], f32, tag="sc")
                        nc.tensor.matmul(
                            sp[:, 0:512],
                            lhsT=kt_sb[:, kv, kt0 * 128:(kt0 + 1) * 128],
                            rhs=qt_sb[:, pair, :, q0:q0 + 256],
                            start=True, stop=True)
                        if diag:
                            nc.tensor.matmul(
                                sp[:, 512:768],
                                lhsT=kt_sb[:, kv, kt1 * 128:(kt1 + 1) * 128],
                                rhs=qt_sb[:, pair, :, q0 + 128:q0 + 256],
                                start=True, stop=True)
                        else:
                            nc.tensor.matmul(
                                sp[:, 512:1024],
                                lhsT=kt_sb[:, kv, kt1 * 128:(kt1 + 1) * 128],
                                rhs=qt_sb[:, pair, :, q0:q0 + 256],
                                start=True, stop=True)
                        pt = ptp.tile([128, 1024], bf, tag="pt")
                        ncols = 768 if diag else 1024
                        nc.scalar.activation(pt[:, 0:ncols], sp[:, 0:ncols],
                                             AF.Exp, scale=1.0 / np.sqrt(HD))
                        if diag:
                            pe = pt[:, 0:512].rearrange("p (h q) -> p h q", h=2)
                            nc.vector.tensor_mul(pe[:, :, 0:128],
                                                 pe[:, :, 0:128], mth_sb[:])
                            po = pt[:, 512:768].rearrange(
                                "p (h q) -> p h q", h=2)
                            nc.vector.tensor_mul(po[:], po[:], mth_sb[:])
                        dripn(2)
                        nc.tensor.matmul(
                            otps[0:65, :, :],
                            lhsT=vone_sb[:, kt0, 65 * kv:65 * kv + 65],
                            rhs=pt[:, 0:512],
                            start=first, stop=False,
                            skip_group_check=True)
                        if diag:
                            for h in range(2):
                                nc.tensor.matmul(
                                    otps[0:65, h, 128:256],
                                    lhsT=vone_sb[:, kt1, 65 * kv:65 * kv + 65],
                                    rhs=pt[:, 512 + h * 128:640 + h * 128],
                                    start=False, stop=(last and h == 1),
                                    skip_group_check=True)
                        else:
                            nc.tensor.matmul(
                                otps[0:65, :, :],
                                lhsT=vone_sb[:, kt1, 65 * kv:65 * kv + 65],
                                rhs=pt[:, 512:1024],
                                start=False, stop=last,
                                skip_group_check=True)
                    otps = otps[:, :, :].rearrange("p h q -> p (h q)")
                else:
                    otps = pso.tile([128, 512], f32, tag="otp")
                    for idx, (kt, mi) in enumerate(kts):
                        spb = pss.tile([128, 1024], f32, tag="sc")
                        sp = spb[:, 0:512]
                        nc.tensor.matmul(
                            sp,
                            lhsT=kt_sb[:, kv, kt * 128:(kt + 1) * 128],
                            rhs=qt_sb[:, pair, :, q0:q0 + 256],
                            start=True, stop=True)
                        pt = ptp.tile([128, 1024], bf, tag="pt")
                        nc.scalar.activation(pt[:, 0:512], sp, AF.Exp,
                                             scale=1.0 / np.sqrt(HD))
                        if mi is not None:
                            nc.vector.tensor_mul(pt[:, 0:512], pt[:, 0:512],
                                                 mt_sb[:, mi, :])
                        dripn(1)
                        nc.tensor.matmul(
                            otps[0:65, :],
                            lhsT=vone_sb[:, kt, 65 * kv:65 * kv + 65],
                            rhs=pt[:, 0:512],
                            start=(idx == 0), stop=(idx == len(kts) - 1))
                # normalization (off the PE critical path); a single [0:65]
                # copy frees the psum bank in one DVE op
                orw = bcp.tile([65, 512], bf, tag="orw")
                nc.vector.tensor_copy(orw[:], otps[0:65, :])
                rs1 = rip.tile([1, 512], f32, tag="rs1")
                nc.vector.tensor_copy(rs1[:], orw[64:65, :])
                ri = rip.tile([1, 512], f32, tag="ri")
                nc.vector.reciprocal_approx_fast(ri[:], rs1[:])
                rib = rip.tile([1, 512], bf, tag="rib")
                nc.vector.tensor_copy(rib[:], ri[:])
                bcsf = bcp.tile([64, 512], bf, tag="bc")
                nc.gpsimd.partition_broadcast(bcsf[:], rib[:])
                qo = q0 - s0
                ot_t = ot_ts[st]
                nc.vector.tensor_mul(ot_t[0:64, pair, qo:qo + 256],
                                     orw[0:64, 0:256], bcsf[:, 0:256])
                nc.vector.tensor_mul(ot_t[64:128, pair, qo:qo + 256],
                                     orw[0:64, 256:512], bcsf[:, 256:512])
                return done_in_stripe

            # ---------- emission ----------
            # Stripe-0 head: K/V/Q0 proj chunks first, then qi=0 attention
            # pairs woven between the remaining q-pair proj chunks so exp
            # streaming starts while proj still owns the PE.
            _, p0 = proj_thunks(0, with_dma=False)
            ot_ts[0] = otp.tile([128, 4, 512], bf, tag="ot_t", name="ot_t0")
            for t in p0[4] + p0[5] + p0[0]:
                t()
            n0 = 0
            for pair in range(4):
                if pair + 1 <= 3:
                    for t in p0[pair + 1]:
                        t()
                n0 = emit_attn_pair(0, 0, pair, n0, tick=False)
            dr.lazy += wo_half_thunks(0, 0)
            for st in range(NST):
                if st + 1 < NST:
                    ths, _ = proj_thunks(st + 1)
                    dr.urgent += ths
                if st > 0:
                    ot_ts[st] = otp.tile([128, 4, 512], bf, tag="ot_t",
                                         name="ot_t")
                    n = 0
                    for pair in range(4):
                        n = emit_attn_pair(st, 2 * st, pair, n)
                    dr.lazy += wo_half_thunks(st, 0)
                else:
                    n = n0
                for pair in range(4):
                    n = emit_attn_pair(st, 2 * st + 1, pair, n)
                dr.lazy += wo_half_thunks(st, 1)
            dr.drain()
    nc.compile()
    return nc


def kernel(x, wq, wk, wv, wo, freqs, mask, start_pos):
    sys.path.insert(0, "/opt/trn_rl_repo")
    from concourse.bass_utils import run_bass_kernel_spmd

    x = np.asarray(x, dtype=np.float32)
    per_core, sched, U = _host_prepare(
        x, np.asarray(wq, np.float32), np.asarray(wk, np.float32),
        np.asarray(wv, np.float32), np.asarray(wo, np.float32),
        np.asarray(freqs, np.float32), np.asarray(mask, np.float32))

    nc = _build_program(sched, U)

    trace = bool(int(os.environ.get("BASSKERNEL_TRACE", "0")))
    if trace and "antenv.axon_hooks" not in sys.modules:
        try:
            import types

            if "/root/.axon_site" not in sys.path:
                sys.path.insert(0, "/root/.axon_site")
            from trn_agent_boot.trn_boot import _ntff_profile_via_ctypes

            _hook = _ntff_profile_via_ctypes("/opt/axon/libaxon_pjrt.so")
            _mod = types.ModuleType("antenv.axon_hooks")
            _mod.get_axon_ntff_profile_hook = lambda: _hook
            _mod.set_axon_ntff_profile_hook = lambda h: None
            sys.modules["antenv.axon_hooks"] = _mod
        except Exception:
            trace = False
    res = run_bass_kernel_spmd(nc, per_core, core_ids=list(range(NCORES)),
                               trace=trace)
    if trace:
        kernel._last_exec_time_ns = res.exec_time_ns
        kernel._last_profile = res.profile_json
    full = np.empty((B, S, D), np.float32)
    for b in range(B):
        acc = res.results[4 * b]["out"].astype(np.float64)
        for g in range(1, 4):
            acc += res.results[4 * b + g]["out"].astype(np.float64)
        full[b] = acc.astype(np.float32)
    return full
